# revision 5
# baseline (speedup 1.0000x reference)
"""ExpertLinear (dense MoE blend) Trainium2 kernel — expert-sharded.

y[b,o] = sum_k ew[b,k] * (x[b,:] @ W[k,o,:]) + sum_k ew[b,k] * bias[k,o]

Sharding: one expert per core (E == 8 == NCORES). Each core computes its
expert's full GEMM z_c = x @ W[c].T for ALL B rows, scales by ew[:, c] on
eviction, and writes a bf16 partial; the host sums the 8 partials and adds
the (tiny) bias term. This reads each expert's weights exactly once
chip-wide: per-core HBM traffic is ~4 MB, and the kernel is PE-bound
(~13.8 us of back-to-back bf16 matmul at 216 ns per [128,128,512]).

Measured structure of a run (core 0 trace): exec_time spans from the
kernel's first instruction (gpsimd entry MEMSET) to the END of the
runtime-appended teardown glue — a fixed ~250-semaphore wipe split across
the five engine sequencers (Tensor's ~52 resets are the largest/slowest
share) plus two ring barriers. That glue is NOT in the NEFF (walrus emits
a 4-instruction tail); the runtime appends it at load, so it cannot be
shrunk — only OVERLAPPED. Hence:

  - NO tile exit barrier at all (see _patch_drain_split): each engine
    falls straight from its last kernel instruction into its glue share
    (glue = per-engine DRAIN, then a serialized ring pass, then the
    resets). The ring order Tensor -> Scalar -> GpSimd -> Vector -> Sync
    guarantees Vector wipes the kernel sems (PE/DVE/ACT/DMAHW lanes) only
    after Scalar's stream — i.e. after the last ACT evict — has retired,
    and Sync's output-DMA data waits are consumed before that. Output
    HBM-write receipts complete under the glue. Starting Tensor's resets
    right after the last matmul also runs them at the still-ramped HAM
    clock instead of the idle-throttled one.
  - Input layout/precision: host packs bf16 [wT | xT] blocks with the
    contraction dim on partitions. Early chunks are small and issued on
    BOTH HWDGE ring groups in parallel (sync: 0a, c23, c46; scalar:
    c12h0, c12h1, c34 — the act-table load only gates scalar's later
    evicts, and DMA issues are emitted before any activation), late
    chunks ride SWDGE (gpsimd, issued at engine boot; its ~3.5 us
    completion-receipt lag is hidden by consumption >= 4 us away).
    Matmul order consumes i0-h0, i1-h0, i1-h1, THEN i0-h1 so the
    SWDGE-delivered wt0-h1 (0b) has receipt margin.
  - Exactly 8 HWDGE DMAs (6 in + yv + ya), one per DMAHW sem lane, so no
    lane-recycle waits; every instruction carries at most one sync wait
    (this walrus build rejects more). Both outputs are issued by sync at
    the end — scalar/ACT retire at their last evict and enter the glue.
  - PSUM: all 8 banks hold the [512, 1024] fp32 partial (4 b-chunks x 2
    o-halves). Banks stop staggered in the last chunk so the DVE (h0) and
    ACT (h1) evictions (x ew, ->bf16) pipeline behind the PE.
  - Zero-matmuls over uninitialized SBUF warm the PE from engine-boot
    until chunk 0a lands so the HAM clock-gate is near 8/8 when real
    matmuls start (bank (0,0)'s start=True clears their garbage).
"""

import numpy as np

B, E, IN, OUT = 512, 8, 1024, 1024
NCORES = 8
P = 128
NIT = IN // P      # 8 i-tiles (contraction chunks)
BT = B // P        # 4 b-chunks (output partition tiles)
NH = OUT // 512    # 2 o-halves (PSUM bank free-dim limit)
CW = OUT + B       # 1536 cols per full i-tile block: wT (1024) + xT (512)
# Warmers must bridge engine-boot to chunk-0a landing (~5.1 us: ~0.6 us
# issue + ~1.7 us DGE start + ~0.5 us transfer + ~2.3 us completion-receipt
# latency) with NO gap: an idle PE re-throttles the HAM clock-gate and the
# next ~7 matmuls run at 2x cost (~1.7 us, measured).
N_DUMMY = 10
EWPAD = 16          # extra bf16 cols on chunk 0a carrying the ew column
AW = 512 + B + EWPAD   # chunk 0a: [wt0 h0 | xT0 | ew]

_compiled = None


def _patch_drain_split():
    """Two deviations from stock TileContext teardown:
    1) the walrus build in this container rejects any instruction carrying
       more than one sync wait, including the kernel-tail Drain that
       TileContext emits with one wait per active semaphore;
    2) the runtime-appended teardown glue (fixed ~250-sem wipe + ring
       barriers, ~6-7 us, measured inside exec_time) begins per-engine as
       soon as that engine's stream retires — so emit NO exit barrier at
       all and let the glue overlap the evict/output tail. The glue's own
       serialized ring pass (Tensor -> Scalar -> GpSimd -> Vector -> Sync)
       provides the cross-engine ordering the barrier used to: Vector,
       which wipes the kernel-sem range, cannot start until Scalar's
       stream (last ACT evict) has retired, and sem increments landing
       after the wipe are re-zeroed by the next execution's entry clear."""
    import concourse.tile as tile_mod

    if getattr(tile_mod.TileContext, "_drain_split_patched", False):
        return

    def _drain_and_barrier(self, tick_clock, wait_clock):
        del tick_clock, wait_clock
        assert self.sems is not None
        popped = self.nc._tile_sem_poison_stack.pop()
        assert popped is self._sem_poison
        # bookkeeping of clear_and_free_semaphores WITHOUT emitting the
        # gpsimd clear + trailing barrier: the next execution's entry
        # sem_clear wipes the kernel sem space anyway, and nothing in
        # this program runs after the engines retire.
        sem_nums = [s.num for s in self.sems.allocated().values()]
        self.nc._state.prepend_free_semaphores(sem_nums)
        for poison_set in self.nc._tile_sem_poison_stack:
            poison_set.update(sem_nums)

    tile_mod.TileContext._drain_and_barrier = _drain_and_barrier
    tile_mod.TileContext._drain_split_patched = True


def _build():
    import concourse.bass as bass
    import concourse.mybir as mybir
    import concourse.tile as tile

    _patch_drain_split()

    f32 = mybir.dt.float32
    bf16 = mybir.dt.bfloat16
    Copy = mybir.ActivationFunctionType.Copy

    nc = bass.Bass()
    # inputs, one dram tensor per DMA chunk
    wx0a_d = nc.dram_tensor("wx0a", [P, AW], bf16, kind="ExternalInput")
    wx0b_d = nc.dram_tensor("wx0b", [P, 512], bf16, kind="ExternalInput")
    c12h0_d = nc.dram_tensor("c12h0", [P, 1024], bf16, kind="ExternalInput")
    c12h1_d = nc.dram_tensor("c12h1", [P, 512], bf16, kind="ExternalInput")
    c23_d = nc.dram_tensor("c23", [P, CW], bf16, kind="ExternalInput")
    c34_d = nc.dram_tensor("c34", [P, CW], bf16, kind="ExternalInput")
    c46_d = nc.dram_tensor("c46", [2 * P, CW], bf16, kind="ExternalInput")
    c68_d = nc.dram_tensor("c68", [2 * P, CW], bf16, kind="ExternalInput")
    yv_d = nc.dram_tensor("yv", [P, BT * 512], bf16, kind="ExternalOutput")
    ya_d = nc.dram_tensor("ya", [P, BT * 512], bf16, kind="ExternalOutput")

    with tile.TileContext(nc) as tc:
        with (
            tc.tile_pool(name="sb", bufs=1) as sb,
            tc.tile_pool(name="ps", bufs=1, space="PSUM") as psp,
        ):
            ewt = sb.tile([P, BT], f32, name="ewt", tag="ewt")
            scr_v = sb.tile([P, 1], f32, name="scrv", tag="scrv")
            scr_s = sb.tile([1, BT], f32, name="scrs", tag="scrs")
            wx0a = sb.tile([P, AW], bf16, name="wx0a", tag="wx0a")
            wx0b = sb.tile([P, 512], bf16, name="wx0b", tag="wx0b")
            c12h0 = sb.tile([P, 1024], bf16, name="c12h0", tag="c12h0")
            c12h1 = sb.tile([P, 512], bf16, name="c12h1", tag="c12h1")
            c23 = sb.tile([P, CW], bf16, name="c23", tag="c23")
            c34 = sb.tile([P, CW], bf16, name="c34", tag="c34")
            c46 = sb.tile([P, 2 * CW], bf16, name="c46", tag="c46")
            c68 = sb.tile([P, 2 * CW], bf16, name="c68", tag="c68")
            y_v = sb.tile([P, BT * 512], bf16, name="yv", tag="yv")
            y_a = sb.tile([P, BT * 512], bf16, name="ya", tag="ya")
            pss = [
                [
                    psp.tile([P, 512], f32, name=f"ps{t}{h}", tag=f"ps{t}{h}")
                    for h in range(NH)
                ]
                for t in range(BT)
            ]

            # HAM warmers: matmuls over (uninitialized) y_v keep the PE
            # array busy from engine-boot until chunk 0a lands. Their
            # garbage lands in bank (0,0), cleared by the real start=True.
            for _ in range(N_DUMMY):
                nc.tensor.matmul(
                    pss[0][0][0:1, :], y_v[:, 0:1], y_v[:, 0:512],
                    start=True, stop=True, skip_group_check=True,
                )

            # SWDGE (gpsimd, issued at boot): wt0-h1 then i-tiles 6,7.
            nc.gpsimd.dma_start(wx0b[:], wx0b_d[:])
            nc.gpsimd.dma_start(
                c68[:].rearrange("p (n c) -> p n c", n=2),
                c68_d[:].rearrange("(n p) c -> p n c", p=P),
            )
            # HWDGE on both ring groups in parallel: sync takes 0a (gates
            # the first matmuls), c23, c46; scalar takes c12h0/h1, c34.
            nc.sync.dma_start(wx0a[:], wx0a_d[:])
            nc.scalar.dma_start(c12h0[:], c12h0_d[:])
            nc.scalar.dma_start(c12h1[:], c12h1_d[:])
            nc.sync.dma_start(c23[:], c23_d[:])
            nc.scalar.dma_start(c34[:], c34_d[:])
            nc.sync.dma_start(
                c46[:].rearrange("p (n c) -> p n c", n=2),
                c46_d[:].rearrange("(n p) c -> p n c", p=P),
            )

            # i0 h0: start banks (t,0); lhsT (xT) and rhs (wT h0) both in
            # 0a -> a single data wait.
            for t in range(BT):
                nc.tensor.matmul(
                    pss[t][0][:], wx0a[:, 512 + P * t:512 + P * (t + 1)],
                    wx0a[:, 0:512],
                    start=True, stop=False, skip_group_check=(t == 0),
                )
            # i1 h0: both operands in c12h0 -> single wait.
            for t in range(BT):
                nc.tensor.matmul(
                    pss[t][0][:], c12h0[:, 512 + P * t:512 + P * (t + 1)],
                    c12h0[:, 0:512],
                    start=False, stop=False, skip_group_check=(t == 0),
                )
            # i1 h1: start banks (t,1); rhs in c12h1 (own wait), lhsT in
            # c12h0 (already absorbed in PE order).
            for t in range(BT):
                nc.tensor.matmul(
                    pss[t][1][:], c12h0[:, 512 + P * t:512 + P * (t + 1)],
                    c12h1[:, 0:512],
                    start=True, stop=False,
                )
            # i0 h1: rhs = 0b (SWDGE; consumed 12 matmuls in -> receipt
            # margin), lhsT in 0a (absorbed).
            for t in range(BT):
                nc.tensor.matmul(
                    pss[t][1][:], wx0a[:, 512 + P * t:512 + P * (t + 1)],
                    wx0b[:, 0:512],
                    start=False, stop=False,
                )
            # remaining i-tiles: chunk-major, bank-major within a chunk so
            # banks stop staggered in the last chunk and the evictions
            # pipeline behind the PE. h1 before h0 within a (t, n): ACT's
            # stops lead DVE's, so ACT's (slower) evicts start earlier.
            chunks = [(c23, [2]), (c34, [3]), (c46, [4, 5]), (c68, [6, 7])]
            for wx, tiles in chunks:
                for t in range(BT):
                    for j, n in enumerate(tiles):
                        off = j * CW
                        lhsT = wx[
                            :, off + OUT + P * t:off + OUT + P * (t + 1)
                        ]
                        for h in (1, 0):
                            nc.tensor.matmul(
                                pss[t][h][:], lhsT,
                                wx[:, off + 512 * h:off + 512 * (h + 1)],
                                start=False,
                                stop=(n == NIT - 1),
                                skip_group_check=(t == 0 and h == 0),
                            )

            # ew rides in chunk 0a as bf16; DVE upconverts it once (also
            # absorbing 0a's wait on the DVE side), and the ACT absorber
            # reads the converted copy so real evictions carry only their
            # PE wait (single-wait limit).
            nc.vector.tensor_copy(ewt[:], wx0a[:, 512 + B:512 + B + BT])
            nc.vector.tensor_scalar_mul(scr_v[:], wx0a[:, 0:1], ewt[:, 0:1])
            nc.scalar.activation(scr_s[:], ewt[0:1, :], Copy)

            # evict: y[b,:] = ps[b,:] * ew[b]; DVE takes h=0, ACT h=1.
            for t in range(BT):
                sc = ewt[:, t:t + 1]
                nc.vector.tensor_scalar_mul(
                    y_v[:, t * 512:(t + 1) * 512], pss[t][0][:], sc
                )
                nc.scalar.activation(
                    y_a[:, t * 512:(t + 1) * 512], pss[t][1][:], Copy, scale=sc
                )
            # yv issued by sync (one data wait on DVE's ticks, lane 7); ya
            # issued by scalar right after its own evicts (no wait at all
            # in ACT program order, lane 8) — every engine reaches the
            # teardown glue's entry ring ASAP after the last matmul, since
            # the ring is a FULL barrier gating the (slow, ~6 us on
            # Tensor) semaphore-wipe. HBM-write receipts complete under
            # the glue.
            nc.sync.dma_start(yv_d[:], y_v[:])
            nc.scalar.dma_start(ya_d[:], y_a[:])

    return nc


def _get_compiled():
    global _compiled
    if _compiled is None:
        _compiled = _build()
    return _compiled


_pack_cache = None


def _make_in_maps(x, expert_weights, weight, bias):
    global _pack_cache
    import ml_dtypes

    bf16 = ml_dtypes.bfloat16
    if _pack_cache is None or _pack_cache[0] is not weight:
        w = np.asarray(weight, dtype=np.float32)
        per_core = []
        for c in range(NCORES):
            wT = w[c].T.reshape(NIT, P, OUT).astype(bf16)  # [p,o]=W[c,o,128n+p]
            a0 = np.zeros((P, AW), dtype=bf16)
            a0[:, :512] = wT[0, :, :512]
            b0 = np.ascontiguousarray(wT[0, :, 512:])
            c12h0 = np.zeros((P, 1024), dtype=bf16)
            c12h0[:, :512] = wT[1, :, :512]
            c12h1 = np.ascontiguousarray(wT[1, :, 512:])
            c23 = np.zeros((P, CW), dtype=bf16)
            c23[:, :OUT] = wT[2]
            c34 = np.zeros((P, CW), dtype=bf16)
            c34[:, :OUT] = wT[3]
            c46 = np.zeros((2, P, CW), dtype=bf16)
            c46[:, :, :OUT] = wT[4:6]
            c68 = np.zeros((2, P, CW), dtype=bf16)
            c68[:, :, :OUT] = wT[6:8]
            per_core.append((a0, b0, c12h0, c12h1, c23, c34, c46, c68))
        _pack_cache = (weight, per_core)
    _, per_core = _pack_cache

    x = np.asarray(x, dtype=np.float32)
    ew = np.asarray(expert_weights, dtype=np.float32)
    # xT tile n: [p, b] = x[b, 128n+p]
    xTb = x.T.reshape(NIT, P, B).astype(bf16)
    in_maps = []
    for c in range(NCORES):
        a0, b0, c12h0, c12h1, c23, c34, c46, c68 = per_core[c]
        a0[:, 512:512 + B] = xTb[0]
        a0[:, 512 + B:512 + B + BT] = ew[:, c].reshape(BT, P).T.astype(bf16)
        c12h0[:, 512:] = xTb[1]
        c23[:, OUT:] = xTb[2]
        c34[:, OUT:] = xTb[3]
        c46[:, :, OUT:] = xTb[4:6]
        c68[:, :, OUT:] = xTb[6:8]
        in_maps.append({
            "wx0a": a0,
            "wx0b": b0,
            "c12h0": c12h0,
            "c12h1": c12h1,
            "c23": c23,
            "c34": c34,
            "c46": c46.reshape(2 * P, CW),
            "c68": c68.reshape(2 * P, CW),
        })
    return in_maps


def kernel(x, expert_weights, weight, bias, _trace=False):
    from concourse.bass_utils import run_bass_kernel_spmd

    nc = _get_compiled()
    in_maps = _make_in_maps(x, expert_weights, weight, bias)
    res = run_bass_kernel_spmd(
        nc, in_maps, core_ids=list(range(NCORES)), trace=_trace
    )
    acc = np.zeros((B, OUT), dtype=np.float32)
    for r in res.results:
        # yv[p, t*512+j] = y[128t+p, j]; ya[p, t*512+j] = y[128t+p, 512+j]
        yv = np.asarray(r["yv"], dtype=np.float32).reshape(P, BT, 512)
        ya = np.asarray(r["ya"], dtype=np.float32).reshape(P, BT, 512)
        acc[:, :512] += yv.transpose(1, 0, 2).reshape(B, 512)
        acc[:, 512:] += ya.transpose(1, 0, 2).reshape(B, 512)
    ew = np.asarray(expert_weights, dtype=np.float32)
    b = np.asarray(bias, dtype=np.float32)
    y = acc + ew @ b
    if _trace:
        return y, res
    return y


# revision 6
# speedup vs baseline: 1.1069x; 1.1069x over previous
"""ExpertLinear (dense MoE blend) Trainium2 kernel — expert-sharded.

y[b,o] = sum_k ew[b,k] * (x[b,:] @ W[k,o,:]) + sum_k ew[b,k] * bias[k,o]

Sharding: one expert per core (E == 8 == NCORES). Each core computes its
expert's full GEMM z_c = x @ W[c].T for ALL B rows, scales by ew[:, c] on
eviction, and writes a bf16 partial; the host sums the 8 partials and adds
the (tiny) bias term. This reads each expert's weights exactly once
chip-wide: per-core HBM traffic is ~4 MB (vs ~18.5 MB for data-parallel).

Measured reality this schedule is tuned against (core-0 traces):
  - exec_time spans from the kernel's first instruction (gpsimd entry
    MEMSET) to the END of the runtime-appended teardown glue. The glue is
    NOT in the NEFF (walrus emits a 4-instruction tail); the runtime
    appends, per engine: DRAIN -> a FULL-barrier entry ring -> its share
    of a fixed ~250-semaphore wipe (Tensor's ~52 resets at ~115 ns are
    the largest/slowest share, ~6 us) -> exit ring -> NOTIFY. It cannot
    be shrunk, only overlapped/entered sooner.
  - All HWDGE input DMAs stripe over the SAME 16 chip queues, so arrival
    order == issue order and the stream is bandwidth-paced (~2.2-2.5
    TB/s chip-wide for 8 cores x 3 MB). Issuing chunks on other paths
    (scalar's ring group, SWDGE) makes them RACE the sync-issued stream
    for HBM and starves later chunks — keep every input on sync's FIFO
    (plus 0b on SWDGE, which is small and needed early). The matmul
    phase below is DMA-arrival-paced, not PE-paced, until ~i-tile 4; the
    measured floor for the first real matmul is ~5.1 us (issue ~0.6 +
    DGE start ~1.7 + transfer + ~2.3 us completion-receipt latency).
  - An idle PE re-throttles the HAM clock-gate (next ~7 matmuls run at
    ~2x cost): the N_DUMMY warmers must bridge boot -> chunk-0a landing
    with no gap, and chunk margins must prevent mid-phase stalls.

Layout/precision:
  - Host packs per-i-tile blocks [wT tile n | xT tile n] (bf16,
    contraction dim on partitions). I-tile 0 is split across the two DGE
    paths: 0a (HWDGE) = [wt0-h0 | full x tile | ew] feeds the first four
    matmuls; 0b (SWDGE, issued by gpsimd at engine boot) = wt0-h1 only,
    needed four matmuls later — outside SWDGE's slow (~3.5us) completion
    receipt. I-tiles 1-7 stream as 4 HWDGE chunks sized [1,1,2,3].
  - Exactly 8 HWDGE DMAs (6 in, yv + ya out), one per DMAHW sem lane, so
    no DMA carries a lane-recycle wait on top of its data wait (this
    walrus build rejects >1 sync wait per instruction). The same limit
    shapes the evict phase: ewt's bf16->f32 upconvert on DVE plus one
    tensor_scalar read-absorber and one ACT absorber keep every
    instruction at a single wait.
  - NO tile exit barrier at all (see _patch_drain_split): each engine
    falls straight from its last kernel instruction into the glue, whose
    own entry ring provides the ordering the barrier used to. The ring
    order (Tensor -> Scalar -> GpSimd -> Vector -> Sync wipe blocks)
    means Vector wipes the kernel sems only after Scalar's stream (last
    ACT evict + ya issue) retired, and Sync's output data waits are
    consumed before that. Output HBM-write receipts and any late sem
    increments complete under the glue / are re-zeroed by the next
    execution's entry clear.
  - PSUM: all 8 banks hold the [512, 1024] fp32 partial (4 b-chunks x 2
    o-halves). Accumulation is chunk-major/bank-major, with h1 BEFORE h0
    inside each (t, n) of the last chunk so ACT's (slower) evictions
    start one matmul earlier; banks complete staggered and the DVE/ACT
    evictions (x ew, ->bf16) pipeline behind the PE. yv ships via sync,
    ya via scalar right after its own evicts — every engine reaches the
    glue's entry ring ASAP after the last matmul.
"""

import numpy as np

B, E, IN, OUT = 512, 8, 1024, 1024
NCORES = 8
P = 128
NIT = IN // P      # 8 i-tiles (contraction chunks)
BT = B // P        # 4 b-chunks (output partition tiles)
NH = OUT // 512    # 2 o-halves (PSUM bank free-dim limit)
CW = OUT + B  # 1536 cols per i-tile block: wT tile (1024) + xT tile (512)
XOFF = OUT          # x region offset inside an i-tile block
N_DUMMY = 8
EWPAD = 16          # extra bf16 cols on chunk 0a carrying the ew column
A_XC = 512          # chunk 0a carries the FULL x tile: 4 h0 matmuls run
AW = 512 + A_XC + EWPAD   # before 0b (SWDGE, slow receipt) is needed
BW = 512                  # chunk 0b: [wt0 h1] only
# i-tile ranges per DMA chunk: fine-grained early chunks keep every
# chunk's completion semaphore ahead of the PE even when all 8 cores
# contend for HBM (a stall also re-throttles the HAM clock-gate, which
# costs 2-3 us extra — margins prevent it).
CHUNKS = [(0, 1), (1, 2), (2, 3), (3, 4), (4, 6), (6, 8)]

_compiled = None


def _patch_drain_split():
    """Suppress TileContext's kernel-tail teardown entirely:
    1) the walrus build in this container rejects any instruction carrying
       more than one sync wait, including the multi-wait Drain TileContext
       emits;
    2) the runtime-appended teardown glue (fixed ~250-sem wipe behind a
       full entry ring/barrier, ~6-7 us, measured inside exec_time) begins
       only after every engine retires — an exit barrier would only delay
       that. The glue's serialized wipe order means the kernel-sem range
       is wiped only after Scalar's stream retired, which is after all
       PSUM reads; sem increments landing after the wipe are re-zeroed by
       the next execution's entry clear."""
    import concourse.tile as tile_mod

    if getattr(tile_mod.TileContext, "_drain_split_patched", False):
        return

    def _drain_and_barrier(self, tick_clock, wait_clock):
        del tick_clock, wait_clock
        assert self.sems is not None
        popped = self.nc._tile_sem_poison_stack.pop()
        assert popped is self._sem_poison
        # bookkeeping of clear_and_free_semaphores WITHOUT emitting the
        # gpsimd clear + trailing barrier.
        sem_nums = [s.num for s in self.sems.allocated().values()]
        self.nc._state.prepend_free_semaphores(sem_nums)
        for poison_set in self.nc._tile_sem_poison_stack:
            poison_set.update(sem_nums)

    tile_mod.TileContext._drain_and_barrier = _drain_and_barrier
    tile_mod.TileContext._drain_split_patched = True


def _build():
    import concourse.bass as bass
    import concourse.mybir as mybir
    import concourse.tile as tile

    _patch_drain_split()

    f32 = mybir.dt.float32
    bf16 = mybir.dt.bfloat16
    Copy = mybir.ActivationFunctionType.Copy

    nc = bass.Bass()
    # chunk 0 split across the two DGE paths: 0a via HWDGE, 0b via SWDGE
    # (gpsimd issues it at engine-boot, and its DMASW sem lane is outside
    # the DMAHW budget)
    wx0a_d = nc.dram_tensor("wx0a", [P, AW], bf16, kind="ExternalInput")
    wx0b_d = nc.dram_tensor("wx0b", [P, BW], bf16, kind="ExternalInput")
    wxr_d = nc.dram_tensor(
        "wxr", [(NIT - 1) * P, CW], bf16, kind="ExternalInput"
    )
    yv_d = nc.dram_tensor("yv", [P, BT * 512], bf16, kind="ExternalOutput")
    ya_d = nc.dram_tensor("ya", [P, BT * 512], bf16, kind="ExternalOutput")

    with tile.TileContext(nc) as tc:
        with (
            tc.tile_pool(name="sb", bufs=1) as sb,
            tc.tile_pool(name="ps", bufs=1, space="PSUM") as psp,
        ):
            ewt = sb.tile([P, BT], f32, name="ewt", tag="ewt")
            scr_v = sb.tile([P, 1], f32, name="scrv", tag="scrv")
            scr_s = sb.tile([1, BT], f32, name="scrs", tag="scrs")
            wx0a = sb.tile([P, AW], bf16, name="wx0a", tag="wx0a")
            wx0b = sb.tile([P, BW], bf16, name="wx0b", tag="wx0b")
            wxs = [
                sb.tile([P, (e - s) * CW], bf16, name=f"wx{ci}", tag=f"wx{ci}")
                for ci, (s, e) in enumerate(CHUNKS[1:], start=1)
            ]
            y_v = sb.tile([P, BT * 512], bf16, name="yv", tag="yv")
            y_a = sb.tile([P, BT * 512], bf16, name="ya", tag="ya")
            pss = [
                [
                    psp.tile([P, 512], f32, name=f"ps{t}{h}", tag=f"ps{t}{h}")
                    for h in range(NH)
                ]
                for t in range(BT)
            ]

            # HAM warmers: matmuls over (uninitialized) y_v keep the PE
            # array busy from engine-boot until the first chunk lands, so
            # the clock-gate reaches 8/8 before the real matmuls start.
            # Their garbage output lands in bank (0,0), which the real
            # group's start=True clears.
            for _ in range(N_DUMMY):
                nc.tensor.matmul(
                    pss[0][0][0:1, :], y_v[:, 0:1], y_v[:, 0:512],
                    start=True, stop=True, skip_group_check=True,
                )

            # exactly 8 HWDGE DMAs in the whole kernel -> each DMAHW lane
            # is used once, so no DMA ever needs a lane-recycle wait on
            # top of its data wait (single-wait limit). wx0 first so the
            # PE's first real group is gated only by it; all inputs on
            # sync's ring group so queue-FIFO order == consumption order.
            nc.gpsimd.dma_start(wx0b[:], wx0b_d[:])
            nc.sync.dma_start(wx0a[:], wx0a_d[:])
            for ci, (s, e) in enumerate(CHUNKS[1:], start=1):
                src = wxr_d[(s - 1) * P:(e - 1) * P, :].rearrange(
                    "(n p) c -> p n c", p=P
                )
                dst = wxs[ci - 1][:].rearrange("p (n c) -> p n c", n=e - s)
                nc.sync.dma_start(dst, src)

            # i-tile 0: lhsT for all t and rhs h0 live in 0a; rhs h1 in
            # 0b. Order so the first four matmuls are gated only by 0a
            # and the first h1 matmul carries the single 0b wait.
            def _lhsT0(t):
                return wx0a[:, 512 + P * t:512 + P * (t + 1)]

            for t in range(BT):
                nc.tensor.matmul(
                    pss[t][0][:], _lhsT0(t), wx0a[:, 0:512],
                    start=True, stop=False,
                    skip_group_check=(t == 0),
                )
            for t in range(BT):
                nc.tensor.matmul(
                    pss[t][1][:], _lhsT0(t), wx0b[:, 0:512],
                    start=True, stop=False,
                )
            # remaining i-tiles: chunk-major so a group waits only on its
            # chunk's DMA; within a chunk, bank-major with h1 before h0
            # so in the last chunk ACT's stops lead DVE's and the
            # evictions pipeline behind the PE instead of serializing
            # after it.
            for ci, (s, e) in enumerate(CHUNKS[1:], start=1):
                wx = wxs[ci - 1]
                for t in range(BT):
                    for n in range(s, e):
                        off = (n - s) * CW
                        lhsT = wx[
                            :, off + XOFF + P * t:off + XOFF + P * (t + 1)
                        ]
                        for h in (1, 0):
                            nc.tensor.matmul(
                                pss[t][h][:], lhsT,
                                wx[:, off + 512 * h:off + 512 * (h + 1)],
                                start=False,
                                stop=(n == e - 1 and ci == len(CHUNKS) - 1),
                                skip_group_check=(t == 0 and h == 0),
                            )

            # ew rides in chunk 0 as bf16; DVE upconverts it once (this
            # also absorbs the chunk-0 DMA wait for DVE), and the ACT
            # absorber reads the converted copy so real evictions carry
            # only their PE wait (single-wait limit)
            nc.vector.tensor_copy(ewt[:], wx0a[:, 512 + A_XC:512 + A_XC + BT])
            # absorber: reads ewt through the tensor_scalar ptr path so the
            # real DVE evicts don't carry a second (DVE-seq) wait
            nc.vector.tensor_scalar_mul(scr_v[:], wx0a[:, 0:1], ewt[:, 0:1])
            nc.scalar.activation(scr_s[:], ewt[0:1, :], Copy)

            # evict: y[b,:] = ps[b,:] * ew[b]; DVE takes h=0, ACT h=1.
            for t in range(BT):
                sc = ewt[:, t:t + 1]
                nc.vector.tensor_scalar_mul(
                    y_v[:, t * 512:(t + 1) * 512], pss[t][0][:], sc
                )
                nc.scalar.activation(
                    y_a[:, t * 512:(t + 1) * 512], pss[t][1][:], Copy, scale=sc
                )
            # yv via sync (single DVE data wait), ya via scalar (no wait
            # at all in ACT program order): every engine reaches the
            # glue's entry ring ASAP. HBM-write receipts complete under
            # the glue.
            nc.sync.dma_start(yv_d[:], y_v[:])
            nc.scalar.dma_start(ya_d[:], y_a[:])

    return nc


def _get_compiled():
    global _compiled
    if _compiled is None:
        _compiled = _build()
    return _compiled


_pack_cache = None


def _make_in_maps(x, expert_weights, weight, bias):
    global _pack_cache
    import ml_dtypes

    bf16 = ml_dtypes.bfloat16
    if _pack_cache is None or _pack_cache[0] is not weight:
        w = np.asarray(weight, dtype=np.float32)
        wx0s, wxrs = [], []
        for c in range(NCORES):
            wT = w[c].T.reshape(NIT, P, OUT).astype(bf16)  # [p,o]=W[c,o,128n+p]
            a0 = np.zeros((P, AW), dtype=bf16)
            a0[:, :512] = wT[0, :, :512]
            b0 = np.ascontiguousarray(wT[0, :, 512:])
            ar = np.zeros((NIT - 1, P, CW), dtype=bf16)
            ar[:, :, :OUT] = wT[1:]
            wx0s.append((a0, b0))
            wxrs.append(ar)
        _pack_cache = (weight, wx0s, wxrs)
    _, wx0s, wxrs = _pack_cache

    x = np.asarray(x, dtype=np.float32)
    ew = np.asarray(expert_weights, dtype=np.float32)
    # xT tile n: [p, b] = x[b, 128n+p]
    xTb = x.T.reshape(NIT, P, B).astype(bf16)
    in_maps = []
    for c in range(NCORES):
        a0, b0 = wx0s[c]
        a0[:, 512:512 + A_XC] = xTb[0]
        a0[:, 512 + A_XC:512 + A_XC + BT] = (
            ew[:, c].reshape(BT, P).T.astype(bf16)
        )
        wxrs[c][:, :, XOFF:] = xTb[1:]
        in_maps.append({
            "wx0a": a0,
            "wx0b": b0,
            "wxr": wxrs[c].reshape((NIT - 1) * P, CW),
        })
    return in_maps


def kernel(x, expert_weights, weight, bias, _trace=False):
    from concourse.bass_utils import run_bass_kernel_spmd

    nc = _get_compiled()
    in_maps = _make_in_maps(x, expert_weights, weight, bias)
    res = run_bass_kernel_spmd(
        nc, in_maps, core_ids=list(range(NCORES)), trace=_trace
    )
    acc = np.zeros((B, OUT), dtype=np.float32)
    for r in res.results:
        # yv[p, t*512+j] = y[128t+p, j]; ya[p, t*512+j] = y[128t+p, 512+j]
        yv = np.asarray(r["yv"], dtype=np.float32).reshape(P, BT, 512)
        ya = np.asarray(r["ya"], dtype=np.float32).reshape(P, BT, 512)
        acc[:, :512] += yv.transpose(1, 0, 2).reshape(B, 512)
        acc[:, 512:] += ya.transpose(1, 0, 2).reshape(B, 512)
    ew = np.asarray(expert_weights, dtype=np.float32)
    b = np.asarray(bias, dtype=np.float32)
    y = acc + ew @ b
    if _trace:
        return y, res
    return y
